# revision 1
# baseline (speedup 1.0000x reference)
"""Trainium2 Bass kernel for DPMultiheadAttention (L=2048, B=2, E=1024, H=16).

Sharding: batch*head parallel across 8 cores. Core c handles batch c%2 and
heads [4*(c//2), 4*(c//2)+4). Each core computes q/k/v projections for its
256-feature slice, per-head attention, and a partial out-projection; the host
sums the per-batch partials (two partial tensors per core: out_a fp32 carries
pair-0 + bias, out_b bf16 carries pair-1).

The kernel is softmax-bound: the ACT engine's exp stream (128 instructions of
(128,1024), ~137us) is the critical resource; everything else pipelines under
it:
  - All large operands are packed host-side into their exact SBUF layouts so
    every DMA moves multi-KB contiguous rows (the strided-rearrange loads of
    earlier revisions ran at ~55% of DMA bandwidth and starved the ramp).
  - The attention loop runs per (pair, qc, j): one 512-q chunk j-tile emits
    2 score matmuls (128-contraction, zero-padded Q per head), one
    (128,1024) exp, and two 65-wide ctx matmuls (ones-column carries the
    softmax denominators).
  - Projections (V, second-pair Q/K) and the pair-0 out-projection are
    emitted as paced fillers inside the loop (pacing matches expected DMA
    arrival - the PE executes in program order, so an early-emitted matmul
    whose DMA lands late parks the whole PE queue). ctx emission lags until
    its V tile has been emitted for the same reason.
  - Normalization per (pair, qc): reciprocal of the denominator row spread
    over 4 partitions (DVE), broadcast down 64 partitions with a 1-row PE
    matmul against a ones vector (not a DMA - keeps the DMA queue out of the
    critical path), multiply into ctx. The PE/post parts are deferred a few
    iterations so the in-order PE queue never waits on the DVE chain.
  - Output DMAs ride the gpsimd software-DGE queue so they never delay
    input loads on the sync queue.
PSUM: 2 score slots (4 banks) + 2 ctx accumulators (2 banks) + 2 utility
banks (projections, out-projection, normalization broadcast) = 8.
"""

import numpy as np

import concourse.bass as bass
import concourse.tile as tile
from concourse import mybir
from concourse.bass_utils import run_bass_kernel_spmd

L = 2048
B = 2
E = 1024
H = 16
D = 64
NCORES = 8
HPC = H // NCORES * B  # heads per core = 4
FL = HPC * D  # local feature slice = 256
P = 128

BF16 = mybir.dt.bfloat16
FP32 = mybir.dt.float32

TRACE = False
TRACE_KWARGS = {}
LAST_RESULTS = None


class PatchedTileContext(tile.TileContext):
    """This walrus build caps sync-wait slots per instruction at one; Tile's
    sem assigner freely attaches several. Split extra waits onto same-engine
    nops inserted just before the owning instruction."""

    MAX_WAITS = 1

    def _split_inst_waits(self, inst, out_list):
        si = getattr(inst, "sync_info", None)
        if si is not None and len(si.on_wait) > self.MAX_WAITS:
            waits = list(si.on_wait)
            keep = len(waits) - self.MAX_WAITS
            for i in range(0, keep, self.MAX_WAITS):
                out_list.append(
                    mybir.InstNoOp(
                        name=f"I-ws-{self.nc.next_id()}",
                        engine=inst.engine,
                        bass_nofuse=True,
                        sync_info=mybir.SyncInfo(
                            on_wait=waits[i : i + self.MAX_WAITS], on_update=[]
                        ),
                    )
                )
            inst.sync_info = mybir.SyncInfo(
                on_wait=waits[keep:], on_update=list(si.on_update)
            )
        out_list.append(inst)

    def _lower_ordered_insts(self, ordered):
        for insts in ordered.values():
            new_list = []
            for inst in insts:
                self._split_inst_waits(inst, new_list)
            insts[:] = new_list
        super()._lower_ordered_insts(ordered)

    def _drain_and_barrier(self, tick_clock, wait_clock):
        from bass_rust import SyncInfo
        from concourse.vector_clock import ScopedClock

        drain_inst = self.nc.sync.drain()
        wait_clock.add_sem_waits(
            drain_inst.ins, ScopedClock({None: tick_clock.global_clock})
        )
        si = drain_inst.ins.sync_info
        if si is not None and len(si.on_wait) > self.MAX_WAITS:
            waits = list(si.on_wait)
            drain_inst.ins.sync_info = SyncInfo(
                on_wait=waits[: self.MAX_WAITS], on_update=list(si.on_update)
            )
            for i in range(self.MAX_WAITS, len(waits), self.MAX_WAITS):
                nop = self.nc.sync.nop(nofuse=True)
                nop.ins.sync_info = SyncInfo(
                    on_wait=waits[i : i + self.MAX_WAITS], on_update=[]
                )

        self.nc.all_engine_barrier()
        assert self.sems is not None
        popped = self.nc._tile_sem_poison_stack.pop()
        assert popped is self._sem_poison
        self.nc.clear_and_free_semaphores(list(self.sems.allocated().values()))
        self.nc.all_engine_barrier()


def _bcast_ap(t):
    """DRAM 1-D tensor -> (128, len) partition-broadcast AP for DMA."""
    ap = t[:]
    return bass.AP(tensor=ap.tensor, offset=ap.offset, ap=[[0, P], *ap.ap])


KT = E // P  # 8 contraction tiles for projections
MT = FL // P  # 2 feature tiles (= head pairs)
NQ = L // 512  # 4 token chunks of 512
LT = L // P  # 16 token tiles of 128
QC = 4  # q chunks of 512 per pair
VW = 66  # V row width: 64 dims + ones column + pad


def build_nc():
    nc = bass.Bass()

    xq = nc.declare_dram_parameter("xq_t", [P, NQ, KT, 512], BF16, isOutput=False)
    xk = nc.declare_dram_parameter("xk_t", [P, NQ, KT, 512], BF16, isOutput=False)
    xv = nc.declare_dram_parameter("xv_t", [P, NQ, KT, 512], BF16, isOutput=False)
    wq = nc.declare_dram_parameter("wq_t", [P, KT, FL], BF16, isOutput=False)
    wk = nc.declare_dram_parameter("wk_t", [P, KT, FL], BF16, isOutput=False)
    wv = nc.declare_dram_parameter("wv_t", [P, KT, FL], BF16, isOutput=False)
    wo = nc.declare_dram_parameter("wo_t", [P, MT, E], BF16, isOutput=False)
    bq = nc.declare_dram_parameter("bq", [P, MT], FP32, isOutput=False)
    bk = nc.declare_dram_parameter("bk", [P, MT], FP32, isOutput=False)
    bv = nc.declare_dram_parameter("bv", [FL], FP32, isOutput=False)
    bo = nc.declare_dram_parameter("bo", [E], FP32, isOutput=False)
    out_a = nc.declare_dram_parameter("out_a", [L, E], FP32, isOutput=True)
    out_b = nc.declare_dram_parameter("out_b", [L, E], BF16, isOutput=True)

    with PatchedTileContext(nc) as tc:
        with (
            tc.tile_pool(name="singles", bufs=1) as singles,
            tc.tile_pool(name="pt", bufs=12) as pt_pool,
            tc.tile_pool(name="norm", bufs=2) as norm_pool,
            tc.tile_pool(name="outsb", bufs=2) as out_pool,
            tc.tile_pool(name="s_psum", bufs=2, space="PSUM") as s_psum,
            tc.tile_pool(name="c_psum", bufs=2, space="PSUM") as c_psum,
            tc.tile_pool(name="u_psum", bufs=2, space="PSUM") as u_psum,
        ):
            # ---- ACT warm-up: force the exp table load at t~0 ----
            warm_sb = singles.tile([1, 8], FP32, tag="warm")
            nc.vector.memset(warm_sb[:], 0.0)
            warm_o = singles.tile([1, 8], BF16, tag="warmo")
            nc.scalar.activation(
                warm_o[:], warm_sb[:], mybir.ActivationFunctionType.Exp
            )

            # ---- weights / biases / staging ----
            wq_sb = singles.tile([P, KT, FL], BF16, tag="wq")
            wk_sb = singles.tile([P, KT, FL], BF16, tag="wk")
            wv_sb = singles.tile([P, KT, FL], BF16, tag="wv")
            wo_sb = singles.tile([P, MT, E], BF16, tag="wo")
            bq_sb = singles.tile([P, MT], FP32, tag="bq")
            bk_sb = singles.tile([P, MT], FP32, tag="bk")
            bv_sb = singles.tile([P, FL], FP32, tag="bv")
            bo_sb = singles.tile([P, E], FP32, tag="bo")
            ones_sb = singles.tile([1, D], BF16, tag="ones")
            nc.vector.memset(ones_sb[:], 1.0)

            xq_sb = singles.tile([P, NQ, KT, 512], BF16, tag="xq")
            xk_sb = singles.tile([P, NQ, KT, 512], BF16, tag="xk")
            xv_sb = singles.tile([P, NQ, KT, 512], BF16, tag="xv")

            # Q^T zero-padded per head: head h of pair mt lives in partition
            # rows [64*(h%2), 64*(h%2)+64) of qtp[:, h, :]; other rows 0.
            qtp = singles.tile([P, HPC, L], BF16, tag="qtp")
            nc.vector.memset(qtp[:], 0.0)
            kt_sb = singles.tile([P, MT, L], BF16, tag="kt")
            v_sb = singles.tile([P, LT, HPC, VW], BF16, tag="v")
            ctx_sb = singles.tile([P, MT, L], BF16, tag="ctx")
            nc.vector.memset(v_sb[:, :, :, D : D + 1], 1.0)

            # ---- DMA emission: deadline-ordered contiguous chunks ----
            nc.sync.dma_start(wq_sb[:], wq[:])
            nc.sync.dma_start(xq_sb[:, 0], xq[:, 0])
            nc.sync.dma_start(wk_sb[:], wk[:])
            nc.sync.dma_start(bq_sb[:], bq[:])
            nc.sync.dma_start(bk_sb[:], bk[:])
            for nq in range(NQ):
                nc.sync.dma_start(xk_sb[:, nq], xk[:, nq])
            nc.sync.dma_start(wv_sb[:], wv[:])
            nc.sync.dma_start(bv_sb[:], _bcast_ap(bv))
            nc.sync.dma_start(xv_sb[:, 0], xv[:, 0])
            nc.sync.dma_start(xq_sb[:, 1], xq[:, 1])
            nc.sync.dma_start(xv_sb[:, 1], xv[:, 1])
            nc.sync.dma_start(xv_sb[:, 2], xv[:, 2])
            nc.sync.dma_start(xv_sb[:, 3], xv[:, 3])
            nc.sync.dma_start(xq_sb[:, 2], xq[:, 2])
            nc.sync.dma_start(xq_sb[:, 3], xq[:, 3])
            nc.sync.dma_start(wo_sb[:], wo[:])
            nc.sync.dma_start(bo_sb[:], _bcast_ap(bo))

            # ---- projection emitters ----
            def qk_proj(which, mt, nq):
                x_sb, w_sb, b_sb = (
                    (xq_sb, wq_sb, bq_sb)
                    if which == "q"
                    else (xk_sb, wk_sb, bk_sb)
                )
                ps = u_psum.tile([P, 512], FP32, tag="u", name=f"p{which}_{mt}_{nq}")
                for k in range(KT):
                    nc.tensor.matmul(
                        ps[:],
                        w_sb[:, k, bass.ts(mt, P)],
                        x_sb[:, nq, k, :],
                        start=(k == 0),
                        stop=(k == KT - 1),
                    )
                if which == "q":
                    # split per head into the zero-padded layout
                    nc.vector.tensor_scalar_add(
                        qtp[0:D, 2 * mt, bass.ts(nq, 512)],
                        ps[0:D],
                        b_sb[0:D, mt : mt + 1],
                    )
                    nc.vector.tensor_scalar_add(
                        qtp[D:P, 2 * mt + 1, bass.ts(nq, 512)],
                        ps[D:P],
                        b_sb[D:P, mt : mt + 1],
                    )
                else:
                    nc.vector.tensor_scalar_add(
                        kt_sb[:, mt, bass.ts(nq, 512)],
                        ps[:],
                        b_sb[:, mt : mt + 1],
                    )

            def v_proj(lt):
                ps = u_psum.tile([P, 512], FP32, tag="u", name=f"pv_{lt}")
                for k in range(KT):
                    nc.tensor.matmul(
                        ps[:, :FL],
                        xv_sb[:, lt // 4, k, bass.ds((lt % 4) * P, P)],
                        wv_sb[:, k, :],
                        start=(k == 0),
                        stop=(k == KT - 1),
                    )
                nc.vector.tensor_add(
                    v_sb[:, lt, :, 0:D],
                    ps[:, :FL].rearrange("p (h d) -> p h d", d=D),
                    bv_sb.rearrange("p (h d) -> p h d", d=D),
                )

            def out_proj(pair, lt):
                # pair 0 -> out_a fp32 (+bias), pair 1 -> out_b bf16
                pss = []
                for nn in range(2):
                    ps = u_psum.tile(
                        [P, 512], FP32, tag="u", name=f"po{pair}_{lt}_{nn}"
                    )
                    nc.tensor.matmul(
                        ps[:],
                        ctx_sb[:, pair, bass.ts(lt, P)],
                        wo_sb[:, pair, bass.ts(nn, 512)],
                        start=True,
                        stop=True,
                    )
                    pss.append(ps)
                if pair == 0:
                    osb = out_pool.tile([P, E], FP32, tag="oa", name=f"oa_{lt}")
                    for nn in range(2):
                        nc.vector.tensor_add(
                            osb[:, bass.ts(nn, 512)],
                            pss[nn][:],
                            bo_sb[:, bass.ts(nn, 512)],
                        )
                    nc.gpsimd.dma_start(out_a[bass.ts(lt, P), :], osb[:])
                else:
                    osb = out_pool.tile([P, E], BF16, tag="ob", name=f"ob_{lt}")
                    for nn in range(2):
                        nc.vector.tensor_copy(osb[:, bass.ts(nn, 512)], pss[nn][:])
                    nc.gpsimd.dma_start(out_b[bass.ts(lt, P), :], osb[:])

            # ---- pre-attention: Q/K pair0 first token chunk only ----
            qk_proj("q", 0, 0)
            qk_proj("k", 0, 0)

            # ---- filler schedule: (min_iter, v_lt_or_None, fn) ----
            # Paced by expected DMA arrival so an emitted matmul never parks
            # the in-order PE queue on a distant DMA.
            f0 = [
                (1, None, lambda: qk_proj("k", 0, 1)),
                (4, None, lambda: qk_proj("k", 0, 2)),
                (7, None, lambda: qk_proj("k", 0, 3)),
                (13, None, lambda: qk_proj("q", 0, 1)),
            ]
            min_v = [11, 11, 12, 12, 16, 16, 17, 17, 19, 19, 20, 20, 22, 22, 23, 23]
            for lt in range(LT):
                f0.append((min_v[lt], lt, lambda lt=lt: v_proj(lt)))
            f0.append((24, None, lambda: qk_proj("q", 0, 2)))
            f0.append((32, None, lambda: qk_proj("q", 1, 0)))
            f0.append((34, None, lambda: qk_proj("k", 1, 0)))
            f0.append((36, None, lambda: qk_proj("k", 1, 1)))
            f0.append((38, None, lambda: qk_proj("k", 1, 2)))
            f0.append((40, None, lambda: qk_proj("q", 0, 3)))
            f0.append((42, None, lambda: qk_proj("k", 1, 3)))
            f0.append((46, None, lambda: qk_proj("q", 1, 1)))
            f0.append((50, None, lambda: qk_proj("q", 1, 2)))
            f0.append((54, None, lambda: qk_proj("q", 1, 3)))
            f0.sort(key=lambda x: x[0])
            f1 = [
                (68 + 2 * lt, None, lambda lt=lt: out_proj(0, lt))
                for lt in range(LT)
            ]
            fillers = {0: f0, 1: f1}
            v_avail = [False] * LT
            pulls = {0: 0, 1: 0}

            def norm_qc(pair, qc, cps, deferred, due):
                # phase 1 (now, DVE): copy ctx+sums out of PSUM, reciprocal,
                # then broadcast 1/sum down 64 partitions with a step-0-source
                # DMA on the (idle by now) sync queue
                chans = []
                for hh in range(2):
                    craw = norm_pool.tile(
                        [D + 1, 512], FP32, tag="craw",
                        name=f"cr_{pair}_{qc}_{hh}",
                    )
                    nc.vector.tensor_copy(craw[:], cps[hh][:])
                    rt = norm_pool.tile([97, P], FP32, tag="rt")
                    for k4 in range(4):
                        nc.vector.tensor_copy(
                            rt[32 * k4 : 32 * k4 + 1, :],
                            craw[D : D + 1, bass.ts(k4, P)],
                        )
                    nc.vector.reciprocal(rt[:], rt[:])
                    rrow = norm_pool.tile([1, 512], FP32, tag="rrow")
                    for k4 in range(4):
                        nc.vector.tensor_copy(
                            rrow[0:1, bass.ts(k4, P)],
                            rt[32 * k4 : 32 * k4 + 1, :],
                        )
                    rb = norm_pool.tile([D, 512], FP32, tag="rb")
                    rap = rrow[0:1, :]
                    nc.sync.dma_start(
                        out=bass.AP(
                            tensor=rb[:].tensor, offset=rb[:].offset,
                            ap=[rb[:].ap[0], [1, 1], rb[:].ap[1]],
                        ),
                        in_=bass.AP(
                            tensor=rap.tensor, offset=rap.offset,
                            ap=[[1, 1], [0, D], rap.ap[-1]],
                        ),
                    )
                    chans.append((craw, rb))

                def phase2():
                    for hh in range(2):
                        craw, rb = chans[hh]
                        nc.vector.tensor_mul(
                            ctx_sb[
                                D * hh : D * hh + D, pair,
                                bass.ds(qc * 512, 512),
                            ],
                            craw[0:D, :],
                            rb[:],
                        )

                deferred.append((due, phase2))
                if pair == 1:

                    def phase3():
                        for lt in range(4 * qc, 4 * qc + 4):
                            out_proj(1, lt)

                    deferred.append((due + 2, phase3))

            # ---- attention + interleaved fillers ----
            it = 0
            for pair in range(MT):
                flist = fillers[pair]
                pending_ctx = []
                deferred = []
                ctx_done = [0] * QC
                next_norm = [0]
                cps_by_qc = {}

                def emit_ctx(pair, qc, j, pts):
                    cps = cps_by_qc[qc]
                    for hh in range(2):
                        h = 2 * pair + hh
                        nc.tensor.matmul(
                            cps[hh][:],
                            v_sb[:, j, h, 0 : D + 1],
                            pts[:, bass.ts(hh, 512)],
                            start=(j == 0),
                            stop=(j == LT - 1),
                        )
                    ctx_done[qc] += 1

                def pull(force=False):
                    p = pulls[pair]
                    if p < len(flist) and (force or flist[p][0] <= it):
                        _, vlt, fn = flist[p]
                        fn()
                        if vlt is not None:
                            v_avail[vlt] = True
                        pulls[pair] = p + 1
                        return True
                    return False

                def drain(limit, keep=2):
                    # keep a couple of ctx pending so a qc-boundary ctx (which
                    # must wait for the previous qc's PSUM evacuation) is never
                    # emitted right behind the scores that feed the exp stream
                    n = 0
                    while pending_ctx and len(pending_ctx) > keep and n < limit:
                        qc, j, pts = pending_ctx[0]
                        if pair == 0 and not v_avail[j]:
                            break
                        pending_ctx.pop(0)
                        emit_ctx(pair, qc, j, pts)
                        n += 1
                    while next_norm[0] < QC and ctx_done[next_norm[0]] == LT:
                        qc = next_norm[0]
                        # first normalization's broadcast DMA queues behind
                        # the tail of the input stream; defer its consumer
                        due = it + (16 if (pair == 0 and qc == 0) else 3)
                        norm_qc(pair, qc, cps_by_qc[qc], deferred, due)
                        next_norm[0] = qc + 1
                    while deferred and deferred[0][0] <= it:
                        deferred.pop(0)[1]()

                for qc in range(QC):
                    cps_by_qc[qc] = {
                        hh: c_psum.tile(
                            [D + 1, 512], FP32, tag="c",
                            name=f"c_{pair}_{qc}_{hh}",
                        )
                        for hh in range(2)
                    }
                    for j in range(LT):
                        sps = s_psum.tile(
                            [P, 1024], FP32, tag="s", name=f"s_{pair}_{qc}_{j}"
                        )
                        for hh in range(2):
                            nc.tensor.matmul(
                                sps[:, bass.ts(hh, 512)],
                                kt_sb[:, pair, bass.ts(j, P)],
                                qtp[:, 2 * pair + hh, bass.ds(qc * 512, 512)],
                                start=True,
                                stop=True,
                            )
                        pts = pt_pool.tile(
                            [P, 1024], BF16, tag="pt", name=f"pt_{pair}_{qc}_{j}"
                        )
                        nc.scalar.activation(
                            pts[:], sps[:], mybir.ActivationFunctionType.Exp
                        )
                        pending_ctx.append((qc, j, pts))
                        pull()
                        pull()
                        drain(3)
                        it += 1
                # pair end: flush fillers needed by pending ctx, then norms
                while pending_ctx or next_norm[0] < QC or deferred:
                    if (
                        pending_ctx
                        and pair == 0
                        and not v_avail[pending_ctx[0][1]]
                    ):
                        pull(force=True)
                    it += 1
                    drain(LT, keep=0)
            # remaining fillers (pair1 out_a tail, if any)
            while pulls[1] < len(fillers[1]):
                fillers[1][pulls[1]][2]()
                pulls[1] += 1

    return nc


_NC = None


def _get_nc():
    global _NC
    if _NC is None:
        _NC = build_nc()
    return _NC


def _pack_x(x2d):
    # (E, L) -> [128, NQ, KT, 512]: A[p, nq, k, m] = x2d[k*128+p, nq*512+m]
    a = x2d.reshape(KT, P, NQ, 512)
    return np.ascontiguousarray(a.transpose(1, 2, 0, 3))


def _pack_w(w2d, cols):
    # (E, cols) -> [128, KT_rows, cols]
    rows = w2d.shape[0] // P
    a = w2d.reshape(rows, P, cols)
    return np.ascontiguousarray(a.transpose(1, 0, 2))


def kernel(query, key, value, w_in, b_in, w_out, b_out):
    import ml_dtypes

    bf16 = ml_dtypes.bfloat16
    query = np.asarray(query, dtype=np.float32)
    key = np.asarray(key, dtype=np.float32)
    value = np.asarray(value, dtype=np.float32)
    w_in = np.asarray(w_in, dtype=np.float32)
    b_in = np.asarray(b_in, dtype=np.float32)
    w_out = np.asarray(w_out, dtype=np.float32)
    b_out = np.asarray(b_out, dtype=np.float32)

    scale = float(D) ** -0.5
    in_maps = []
    for c in range(NCORES):
        b = c % 2
        g = c // 2
        sl = slice(FL * g, FL * (g + 1))
        wq = w_in[0 * E : 1 * E][sl] * scale  # (256, 1024)
        wk = w_in[1 * E : 2 * E][sl]
        wv = w_in[2 * E : 3 * E][sl]
        in_maps.append(
            {
                "xq_t": _pack_x(query[:, b, :].T.astype(bf16)),
                "xk_t": _pack_x(key[:, b, :].T.astype(bf16)),
                "xv_t": _pack_x(value[:, b, :].T.astype(bf16)),
                "wq_t": _pack_w(wq.T.astype(bf16), FL),
                "wk_t": _pack_w(wk.T.astype(bf16), FL),
                "wv_t": _pack_w(wv.T.astype(bf16), FL),
                "wo_t": _pack_w(w_out[:, sl].T.astype(bf16), E),
                "bq": np.ascontiguousarray(
                    (b_in[0 * E : 1 * E][sl] * scale).reshape(MT, P).T
                ),
                "bk": np.ascontiguousarray(b_in[1 * E : 2 * E][sl].reshape(MT, P).T),
                "bv": np.ascontiguousarray(b_in[2 * E : 3 * E][sl]),
                "bo": b_out if c < 2 else np.zeros_like(b_out),
            }
        )

    nc = _get_nc()
    res = run_bass_kernel_spmd(
        nc, in_maps, list(range(NCORES)), trace=TRACE, **TRACE_KWARGS
    )
    global LAST_RESULTS
    LAST_RESULTS = res

    out = np.zeros((L, B, E), dtype=np.float32)
    for c in range(NCORES):
        out[:, c % 2, :] += res.results[c]["out_a"]
        out[:, c % 2, :] += res.results[c]["out_b"].astype(np.float32)
    return out



# revision 6
# speedup vs baseline: 1.1920x; 1.1920x over previous
"""Trainium2 Bass kernel for DPMultiheadAttention (L=2048, B=2, E=1024, H=16).

Sharding: batch*head parallel across 8 cores. Core c handles batch c%2 and
heads [4*(c//2), 4*(c//2)+4). Each core computes q/k/v projections for its
256-feature slice, per-head attention, and a partial out-projection; the host
sums the per-batch partials (two partial tensors per core: out_a fp32 carries
pair-0 + bias, out_b bf16 carries pair-1).

The kernel is softmax-bound: the ACT engine's exp stream (128 instructions of
(128,1024), ~137us) is the critical resource; everything else pipelines under
it:
  - All large operands are packed host-side into their exact SBUF layouts so
    every DMA moves multi-KB contiguous rows (the strided-rearrange loads of
    earlier revisions ran at ~55% of DMA bandwidth and starved the ramp).
  - The attention loop runs per (pair, qc, j): one 512-q chunk j-tile emits
    2 score matmuls (128-contraction, zero-padded Q per head), one
    (128,1024) exp, and two 65-wide ctx matmuls (ones-column carries the
    softmax denominators).
  - Projections (V, second-pair Q/K) and the pair-0 out-projection are
    emitted as paced fillers inside the loop (pacing matches expected DMA
    arrival - the PE executes in program order, so an early-emitted matmul
    whose DMA lands late parks the whole PE queue). ctx emission lags until
    its V tile has been emitted for the same reason.
  - Normalization per (pair, qc): reciprocal of the denominator row spread
    over 4 partitions (DVE), broadcast down 64 partitions with a 1-row PE
    matmul against a ones vector (not a DMA - keeps the DMA queue out of the
    critical path), multiply into ctx. The PE/post parts are deferred a few
    iterations so the in-order PE queue never waits on the DVE chain.
  - Output DMAs ride the gpsimd software-DGE queue so they never delay
    input loads on the sync queue.
PSUM: 2 score slots (4 banks) + 2 ctx accumulators (2 banks) + 2 utility
banks (projections, out-projection, normalization broadcast) = 8.
"""

import numpy as np

import concourse.bass as bass
import concourse.tile as tile
from concourse import mybir
from concourse.bass_utils import run_bass_kernel_spmd

L = 2048
B = 2
E = 1024
H = 16
D = 64
NCORES = 8
HPC = H // NCORES * B  # heads per core = 4
FL = HPC * D  # local feature slice = 256
P = 128

BF16 = mybir.dt.bfloat16
FP32 = mybir.dt.float32

TRACE = False
TRACE_KWARGS = {}
LAST_RESULTS = None


class PatchedTileContext(tile.TileContext):
    """This walrus build caps sync-wait slots per instruction at one; Tile's
    sem assigner freely attaches several. Split extra waits onto same-engine
    nops inserted just before the owning instruction."""

    MAX_WAITS = 1

    def _split_inst_waits(self, inst, out_list):
        si = getattr(inst, "sync_info", None)
        if si is not None and len(si.on_wait) > self.MAX_WAITS:
            waits = list(si.on_wait)
            keep = len(waits) - self.MAX_WAITS
            for i in range(0, keep, self.MAX_WAITS):
                out_list.append(
                    mybir.InstNoOp(
                        name=f"I-ws-{self.nc.next_id()}",
                        engine=inst.engine,
                        bass_nofuse=True,
                        sync_info=mybir.SyncInfo(
                            on_wait=waits[i : i + self.MAX_WAITS], on_update=[]
                        ),
                    )
                )
            inst.sync_info = mybir.SyncInfo(
                on_wait=waits[keep:], on_update=list(si.on_update)
            )
        out_list.append(inst)

    def _lower_ordered_insts(self, ordered):
        for insts in ordered.values():
            new_list = []
            for inst in insts:
                self._split_inst_waits(inst, new_list)
            insts[:] = new_list
        super()._lower_ordered_insts(ordered)

    def _drain_and_barrier(self, tick_clock, wait_clock):
        from bass_rust import SyncInfo
        from concourse.vector_clock import ScopedClock

        drain_inst = self.nc.sync.drain()
        wait_clock.add_sem_waits(
            drain_inst.ins, ScopedClock({None: tick_clock.global_clock})
        )
        si = drain_inst.ins.sync_info
        if si is not None and len(si.on_wait) > self.MAX_WAITS:
            waits = list(si.on_wait)
            drain_inst.ins.sync_info = SyncInfo(
                on_wait=waits[: self.MAX_WAITS], on_update=list(si.on_update)
            )
            for i in range(self.MAX_WAITS, len(waits), self.MAX_WAITS):
                nop = self.nc.sync.nop(nofuse=True)
                nop.ins.sync_info = SyncInfo(
                    on_wait=waits[i : i + self.MAX_WAITS], on_update=[]
                )

        self.nc.all_engine_barrier()
        assert self.sems is not None
        popped = self.nc._tile_sem_poison_stack.pop()
        assert popped is self._sem_poison
        self.nc.clear_and_free_semaphores(list(self.sems.allocated().values()))
        self.nc.all_engine_barrier()


def _bcast_ap(t):
    """DRAM 1-D tensor -> (128, len) partition-broadcast AP for DMA."""
    ap = t[:]
    return bass.AP(tensor=ap.tensor, offset=ap.offset, ap=[[0, P], *ap.ap])


KT = E // P  # 8 contraction tiles for projections
MT = FL // P  # 2 feature tiles (= head pairs)
NQ = L // 512  # 4 token chunks of 512
LT = L // P  # 16 token tiles of 128
QC = 4  # q chunks of 512 per pair
VW = 66  # V row width: 64 dims + ones column + pad


def build_nc():
    nc = bass.Bass()

    xq = nc.declare_dram_parameter("xq_t", [P, NQ, KT, 512], BF16, isOutput=False)
    xk = nc.declare_dram_parameter("xk_t", [P, NQ, KT, 512], BF16, isOutput=False)
    xv = nc.declare_dram_parameter("xv_t", [P, NQ, KT, 512], BF16, isOutput=False)
    wq = nc.declare_dram_parameter("wq_t", [P, KT, FL], BF16, isOutput=False)
    wk = nc.declare_dram_parameter("wk_t", [P, KT, FL], BF16, isOutput=False)
    wv = nc.declare_dram_parameter("wv_t", [P, KT, FL], BF16, isOutput=False)
    wo = nc.declare_dram_parameter("wo_t", [P, MT, E], BF16, isOutput=False)
    bq = nc.declare_dram_parameter("bq", [P, MT], FP32, isOutput=False)
    bk = nc.declare_dram_parameter("bk", [P, MT], FP32, isOutput=False)
    bv = nc.declare_dram_parameter("bv", [FL], FP32, isOutput=False)
    bo = nc.declare_dram_parameter("bo", [E], FP32, isOutput=False)
    out_a = nc.declare_dram_parameter("out_a", [L, E], FP32, isOutput=True)
    out_b = nc.declare_dram_parameter("out_b", [L, E], BF16, isOutput=True)

    with PatchedTileContext(nc) as tc:
        with (
            tc.tile_pool(name="singles", bufs=1) as singles,
            tc.tile_pool(name="pt", bufs=12) as pt_pool,
            tc.tile_pool(name="norm", bufs=2) as norm_pool,
            tc.tile_pool(name="outsb", bufs=2) as out_pool,
            tc.tile_pool(name="s_psum", bufs=2, space="PSUM") as s_psum,
            tc.tile_pool(name="c_psum", bufs=2, space="PSUM") as c_psum,
            tc.tile_pool(name="u_psum", bufs=2, space="PSUM") as u_psum,
        ):
            # ---- ACT warm-up: force the exp table load at t~0 ----
            warm_sb = singles.tile([1, 8], FP32, tag="warm")
            nc.vector.memset(warm_sb[:], 0.0)
            warm_o = singles.tile([1, 8], BF16, tag="warmo")
            nc.scalar.activation(
                warm_o[:], warm_sb[:], mybir.ActivationFunctionType.Exp
            )

            # ---- weights / biases / staging ----
            wq_sb = singles.tile([P, KT, FL], BF16, tag="wq")
            wk_sb = singles.tile([P, KT, FL], BF16, tag="wk")
            wv_sb = singles.tile([P, KT, FL], BF16, tag="wv")
            wo_sb = singles.tile([P, MT, E], BF16, tag="wo")
            bq_sb = singles.tile([P, MT], FP32, tag="bq")
            bk_sb = singles.tile([P, MT], FP32, tag="bk")
            bv_sb = singles.tile([P, FL], FP32, tag="bv")
            bo_sb = singles.tile([P, E], FP32, tag="bo")
            ones_sb = singles.tile([1, D], BF16, tag="ones")
            nc.vector.memset(ones_sb[:], 1.0)

            xq_sb = singles.tile([P, NQ, KT, 512], BF16, tag="xq")
            xk_sb = singles.tile([P, NQ, KT, 512], BF16, tag="xk")
            xv_sb = singles.tile([P, NQ, KT, 512], BF16, tag="xv")

            # Q^T zero-padded per head: head h of pair mt lives in partition
            # rows [64*(h%2), 64*(h%2)+64) of qtp[:, h, :]; other rows 0.
            qtp = singles.tile([P, HPC, L], BF16, tag="qtp")
            nc.vector.memset(qtp[:], 0.0)
            kt_sb = singles.tile([P, MT, L], BF16, tag="kt")
            v_sb = singles.tile([P, LT, HPC, VW], BF16, tag="v")
            ctx_sb = singles.tile([P, MT, L], BF16, tag="ctx")
            nc.vector.memset(v_sb[:, :, :, D : D + 1], 1.0)

            # ---- DMA emission: deadline-ordered contiguous chunks ----
            nc.sync.dma_start(wq_sb[:], wq[:])
            nc.sync.dma_start(xq_sb[:, 0], xq[:, 0])
            nc.sync.dma_start(wk_sb[:], wk[:])
            nc.sync.dma_start(bq_sb[:], bq[:])
            nc.sync.dma_start(bk_sb[:], bk[:])
            for nq in range(NQ):
                nc.sync.dma_start(xk_sb[:, nq], xk[:, nq])
            nc.sync.dma_start(wv_sb[:], wv[:])
            nc.sync.dma_start(bv_sb[:], _bcast_ap(bv))
            nc.sync.dma_start(xv_sb[:, 0], xv[:, 0])
            nc.sync.dma_start(xq_sb[:, 1], xq[:, 1])
            nc.sync.dma_start(xv_sb[:, 1], xv[:, 1])
            nc.sync.dma_start(xv_sb[:, 2], xv[:, 2])
            nc.sync.dma_start(xv_sb[:, 3], xv[:, 3])
            nc.sync.dma_start(xq_sb[:, 2], xq[:, 2])
            nc.sync.dma_start(xq_sb[:, 3], xq[:, 3])
            nc.sync.dma_start(wo_sb[:], wo[:])
            nc.sync.dma_start(bo_sb[:], _bcast_ap(bo))

            # ---- projection emitters ----
            def qk_proj(which, mt, nq):
                x_sb, w_sb, b_sb = (
                    (xq_sb, wq_sb, bq_sb)
                    if which == "q"
                    else (xk_sb, wk_sb, bk_sb)
                )
                ps = u_psum.tile([P, 512], FP32, tag="u", name=f"p{which}_{mt}_{nq}")
                for k in range(KT):
                    nc.tensor.matmul(
                        ps[:],
                        w_sb[:, k, bass.ts(mt, P)],
                        x_sb[:, nq, k, :],
                        start=(k == 0),
                        stop=(k == KT - 1),
                    )
                if which == "q":
                    # split per head into the zero-padded layout
                    nc.vector.tensor_scalar_add(
                        qtp[0:D, 2 * mt, bass.ts(nq, 512)],
                        ps[0:D],
                        b_sb[0:D, mt : mt + 1],
                    )
                    nc.vector.tensor_scalar_add(
                        qtp[D:P, 2 * mt + 1, bass.ts(nq, 512)],
                        ps[D:P],
                        b_sb[D:P, mt : mt + 1],
                    )
                else:
                    nc.vector.tensor_scalar_add(
                        kt_sb[:, mt, bass.ts(nq, 512)],
                        ps[:],
                        b_sb[:, mt : mt + 1],
                    )

            def v_proj(lt):
                ps = u_psum.tile([P, 512], FP32, tag="u", name=f"pv_{lt}")
                for k in range(KT):
                    nc.tensor.matmul(
                        ps[:, :FL],
                        xv_sb[:, lt // 4, k, bass.ds((lt % 4) * P, P)],
                        wv_sb[:, k, :],
                        start=(k == 0),
                        stop=(k == KT - 1),
                    )
                nc.vector.tensor_add(
                    v_sb[:, lt, :, 0:D],
                    ps[:, :FL].rearrange("p (h d) -> p h d", d=D),
                    bv_sb.rearrange("p (h d) -> p h d", d=D),
                )

            def out_proj(pair, lt):
                # pair 0 -> out_a fp32 (+bias), pair 1 -> out_b bf16
                pss = []
                for nn in range(2):
                    ps = u_psum.tile(
                        [P, 512], FP32, tag="u", name=f"po{pair}_{lt}_{nn}"
                    )
                    nc.tensor.matmul(
                        ps[:],
                        ctx_sb[:, pair, bass.ts(lt, P)],
                        wo_sb[:, pair, bass.ts(nn, 512)],
                        start=True,
                        stop=True,
                    )
                    pss.append(ps)
                if pair == 0:
                    osb = out_pool.tile([P, E], FP32, tag="oa", name=f"oa_{lt}")
                    for nn in range(2):
                        nc.vector.tensor_add(
                            osb[:, bass.ts(nn, 512)],
                            pss[nn][:],
                            bo_sb[:, bass.ts(nn, 512)],
                        )
                    nc.gpsimd.dma_start(out_a[bass.ts(lt, P), :], osb[:])
                else:
                    osb = out_pool.tile([P, E], BF16, tag="ob", name=f"ob_{lt}")
                    for nn in range(2):
                        nc.vector.tensor_copy(osb[:, bass.ts(nn, 512)], pss[nn][:])
                    nc.gpsimd.dma_start(out_b[bass.ts(lt, P), :], osb[:])

            # ---- pre-attention: Q/K pair0 first token chunk only ----
            qk_proj("q", 0, 0)
            qk_proj("k", 0, 0)

            # ---- filler schedule: (min_iter, v_lt_or_None, fn) ----
            # Paced by expected DMA arrival so an emitted matmul never parks
            # the in-order PE queue on a distant DMA.
            f0 = [
                (1, None, lambda: qk_proj("k", 0, 1)),
                (4, None, lambda: qk_proj("k", 0, 2)),
                (7, None, lambda: qk_proj("k", 0, 3)),
                (13, None, lambda: qk_proj("q", 0, 1)),
            ]
            min_v = [11, 11, 12, 12, 16, 16, 17, 17, 19, 19, 20, 20, 22, 22, 23, 23]
            for lt in range(LT):
                f0.append((min_v[lt], lt, lambda lt=lt: v_proj(lt)))
            f0.append((24, None, lambda: qk_proj("q", 0, 2)))
            f0.append((32, None, lambda: qk_proj("q", 1, 0)))
            f0.append((34, None, lambda: qk_proj("k", 1, 0)))
            f0.append((36, None, lambda: qk_proj("k", 1, 1)))
            f0.append((38, None, lambda: qk_proj("k", 1, 2)))
            f0.append((40, None, lambda: qk_proj("q", 0, 3)))
            f0.append((42, None, lambda: qk_proj("k", 1, 3)))
            f0.append((46, None, lambda: qk_proj("q", 1, 1)))
            f0.append((50, None, lambda: qk_proj("q", 1, 2)))
            f0.append((54, None, lambda: qk_proj("q", 1, 3)))
            f0.sort(key=lambda x: x[0])
            f1 = [
                (68 + 2 * lt, None, lambda lt=lt: out_proj(0, lt))
                for lt in range(LT)
            ]
            fillers = {0: f0, 1: f1}
            v_avail = [False] * LT
            pulls = {0: 0, 1: 0}

            def norm_qc(pair, qc, cps, deferred, due):
                # phase 1 (now, DVE): evacuate ctx+den rows from PSUM; one
                # packed reciprocal covers both heads (4 col-chunks x 2 heads
                # spread over partitions); recip rows land in bf16.
                craws = []
                rt = norm_pool.tile([97, 256], FP32, tag="rt")
                rrows = []
                for hh in range(2):
                    craw = norm_pool.tile(
                        [D + 1, 512], FP32, tag="craw",
                        name=f"cr_{pair}_{qc}_{hh}",
                    )
                    nc.vector.tensor_copy(craw[:], cps[hh][:])
                    craws.append(craw)
                # den chunks for both heads packed into one reciprocal:
                # partition 32*k4, col half per head (engine ops must start
                # on partition 0/32/64/96)
                for hh in range(2):
                    for k4 in range(4):
                        nc.vector.tensor_copy(
                            rt[32 * k4 : 32 * k4 + 1, 128 * hh : 128 * hh + P],
                            craws[hh][D : D + 1, bass.ts(k4, P)],
                        )
                nc.vector.reciprocal(rt[:], rt[:])
                for hh in range(2):
                    rrow = norm_pool.tile([1, 512], BF16, tag=f"rrow{hh}")
                    for k4 in range(4):
                        nc.vector.tensor_copy(
                            rrow[0:1, bass.ts(k4, P)],
                            rt[32 * k4 : 32 * k4 + 1, 128 * hh : 128 * hh + P],
                        )
                    rrows.append(rrow)

                rb_cell = []

                def phase1b():
                    # broadcast 1/den down 64 partitions per head with rank-1
                    # PE matmuls (engine-semaphore latency, vs the ~4us
                    # completion lag of a DMA-based broadcast)
                    rb = u_psum.tile([P, 512], FP32, tag="u", name=f"rb_{pair}_{qc}")
                    for hh in range(2):
                        nc.tensor.matmul(
                            rb[D * hh : D * hh + D, :],
                            ones_sb[:],
                            rrows[hh][:],
                            start=True,
                            stop=True,
                        )
                    rb_cell.append(rb)

                def phase2():
                    rb = rb_cell[0]
                    for hh in range(2):
                        nc.vector.tensor_mul(
                            ctx_sb[
                                D * hh : D * hh + D, pair,
                                bass.ds(qc * 512, 512),
                            ],
                            craws[hh][0:D, :],
                            rb[D * hh : D * hh + D, :],
                        )

                deferred.append((due, phase1b))
                deferred.append((due + 1, phase2))
                if pair == 1:

                    def phase3():
                        for lt in range(4 * qc, 4 * qc + 4):
                            out_proj(1, lt)

                    deferred.append((due + 2, phase3))

            # ---- attention + interleaved fillers ----
            it = 0
            for pair in range(MT):
                flist = fillers[pair]
                pending_ctx = []
                deferred = []
                ctx_done = [0] * QC
                next_norm = [0]
                cps_by_qc = {}

                def emit_ctx(pair, qc, j, pts):
                    cps = cps_by_qc[qc]
                    for hh in range(2):
                        h = 2 * pair + hh
                        nc.tensor.matmul(
                            cps[hh][:],
                            v_sb[:, j, h, 0 : D + 1],
                            pts[:, bass.ts(hh, 512)],
                            start=(j == 0),
                            stop=(j == LT - 1),
                        )
                    ctx_done[qc] += 1

                def pull(force=False):
                    p = pulls[pair]
                    if p < len(flist) and (force or flist[p][0] <= it):
                        _, vlt, fn = flist[p]
                        fn()
                        if vlt is not None:
                            v_avail[vlt] = True
                        pulls[pair] = p + 1
                        return True
                    return False

                def drain(limit, keep=2):
                    # keep a couple of ctx pending so a qc-boundary ctx (which
                    # must wait for the previous qc's PSUM evacuation) is never
                    # emitted right behind the scores that feed the exp stream
                    n = 0
                    while pending_ctx and len(pending_ctx) > keep and n < limit:
                        qc, j, pts = pending_ctx[0]
                        if pair == 0 and not v_avail[j]:
                            break
                        pending_ctx.pop(0)
                        emit_ctx(pair, qc, j, pts)
                        n += 1
                    while next_norm[0] < QC and ctx_done[next_norm[0]] == LT:
                        qc = next_norm[0]
                        # phase1b needs the ~4us DVE recip chain done; give it
                        # a few iterations of slack so the PE queue never
                        # head-of-line blocks on it
                        norm_qc(pair, qc, cps_by_qc[qc], deferred, it + 3)
                        next_norm[0] = qc + 1
                    while deferred and deferred[0][0] <= it:
                        deferred.pop(0)[1]()

                for qc in range(QC):
                    cps_by_qc[qc] = {
                        hh: c_psum.tile(
                            [D + 1, 512], FP32, tag="c",
                            name=f"c_{pair}_{qc}_{hh}",
                        )
                        for hh in range(2)
                    }
                    for j in range(LT):
                        sps = s_psum.tile(
                            [P, 1024], FP32, tag="s", name=f"s_{pair}_{qc}_{j}"
                        )
                        for hh in range(2):
                            nc.tensor.matmul(
                                sps[:, bass.ts(hh, 512)],
                                kt_sb[:, pair, bass.ts(j, P)],
                                qtp[:, 2 * pair + hh, bass.ds(qc * 512, 512)],
                                start=True,
                                stop=True,
                            )
                        pts = pt_pool.tile(
                            [P, 1024], BF16, tag="pt", name=f"pt_{pair}_{qc}_{j}"
                        )
                        nc.scalar.activation(
                            pts[:], sps[:], mybir.ActivationFunctionType.Exp
                        )
                        pending_ctx.append((qc, j, pts))
                        pull()
                        pull()
                        drain(3)
                        it += 1
                # pair end: flush fillers needed by pending ctx, then norms
                while pending_ctx or next_norm[0] < QC or deferred:
                    if (
                        pending_ctx
                        and pair == 0
                        and not v_avail[pending_ctx[0][1]]
                    ):
                        pull(force=True)
                    it += 1
                    drain(LT, keep=0)
            # remaining fillers (pair1 out_a tail, if any)
            while pulls[1] < len(fillers[1]):
                fillers[1][pulls[1]][2]()
                pulls[1] += 1

    return nc


_NC = None


def _get_nc():
    global _NC
    if _NC is None:
        _NC = build_nc()
    return _NC


def _pack_x(x2d):
    # (E, L) -> [128, NQ, KT, 512]: A[p, nq, k, m] = x2d[k*128+p, nq*512+m]
    a = x2d.reshape(KT, P, NQ, 512)
    return np.ascontiguousarray(a.transpose(1, 2, 0, 3))


def _pack_w(w2d, cols):
    # (E, cols) -> [128, KT_rows, cols]
    rows = w2d.shape[0] // P
    a = w2d.reshape(rows, P, cols)
    return np.ascontiguousarray(a.transpose(1, 0, 2))


def kernel(query, key, value, w_in, b_in, w_out, b_out):
    import ml_dtypes

    bf16 = ml_dtypes.bfloat16
    query = np.asarray(query, dtype=np.float32)
    key = np.asarray(key, dtype=np.float32)
    value = np.asarray(value, dtype=np.float32)
    w_in = np.asarray(w_in, dtype=np.float32)
    b_in = np.asarray(b_in, dtype=np.float32)
    w_out = np.asarray(w_out, dtype=np.float32)
    b_out = np.asarray(b_out, dtype=np.float32)

    scale = float(D) ** -0.5
    in_maps = []
    for c in range(NCORES):
        b = c % 2
        g = c // 2
        sl = slice(FL * g, FL * (g + 1))
        wq = w_in[0 * E : 1 * E][sl] * scale  # (256, 1024)
        wk = w_in[1 * E : 2 * E][sl]
        wv = w_in[2 * E : 3 * E][sl]
        in_maps.append(
            {
                "xq_t": _pack_x(query[:, b, :].T.astype(bf16)),
                "xk_t": _pack_x(key[:, b, :].T.astype(bf16)),
                "xv_t": _pack_x(value[:, b, :].T.astype(bf16)),
                "wq_t": _pack_w(wq.T.astype(bf16), FL),
                "wk_t": _pack_w(wk.T.astype(bf16), FL),
                "wv_t": _pack_w(wv.T.astype(bf16), FL),
                "wo_t": _pack_w(w_out[:, sl].T.astype(bf16), E),
                "bq": np.ascontiguousarray(
                    (b_in[0 * E : 1 * E][sl] * scale).reshape(MT, P).T
                ),
                "bk": np.ascontiguousarray(b_in[1 * E : 2 * E][sl].reshape(MT, P).T),
                "bv": np.ascontiguousarray(b_in[2 * E : 3 * E][sl]),
                "bo": b_out if c < 2 else np.zeros_like(b_out),
            }
        )

    nc = _get_nc()
    res = run_bass_kernel_spmd(
        nc, in_maps, list(range(NCORES)), trace=TRACE, **TRACE_KWARGS
    )
    global LAST_RESULTS
    LAST_RESULTS = res

    out = np.zeros((L, B, E), dtype=np.float32)
    for c in range(NCORES):
        out[:, c % 2, :] += res.results[c]["out_a"]
        out[:, c % 2, :] += res.results[c]["out_b"].astype(np.float32)
    return out

